# revision 34
# baseline (speedup 1.0000x reference)
"""Trainium2 Bass kernel for nn_LongTermMemoryMLP.

Per-batch-weight 3-layer MLP:
    h0 = relu(q @ W0^T + b0); h1 = relu(h0 @ W1^T + b1); out = h1 @ W2^T + b2
with q: [B,S,DIN], W0: [B,DH,DIN], W1: [B,DH,DH], W2: [B,DOUT,DH], B=8.

Sharding: data-parallel over batch — one batch sample (and its weight slabs)
per NeuronCore, 8 cores, no cross-core communication.

Device-side strategy: activations are kept feature-major ([feature, seq],
feature on partitions) so every layer is a plain accumulated matmul with the
(pre-transposed) weights as the stationary operand and the activations as the
moving operand — no on-chip transposes. The final layer flips orientation
(stationary = activation tile, moving = W2^T) so the output lands seq-major
and can be DMA'd out contiguously. Inputs are pre-transposed AND pre-cast to
bf16 on the host: bf16 streams at the PE's full 1 row/cycle (518 cycles
measured per 128x128x512 matmul, the warm roofline) and halves all input DMA
traffic, which bounds the startup ramp. Weights and each seq-chunk of the
query load as single ~0.5-1 MiB DMAs (small transfers run at <50% DMA
efficiency; ~1 MiB runs at ~80%). The output is stored bf16 (halves the
output DMA and the strictly-serial tail after the last matmul) and widened
to fp32 on the host. Accumulation stays fp32 in PSUM; measured end-to-end
relative error is 4.7e-3 against the fp32 reference, vs the 2e-2 gate.

Measured on hardware (runs whose DVFS state is 2.4 GHz): ~241 us total =
~8 us engine preamble + ~225 us PE busy (1024 matmuls at the 216 ns
N=512 warm roofline, plus 15 clock-ramp warmup matmuls that hide under
the startup DMA) + ~3 us residual DMA-wait + ~5 us add/DMA/drain tail.
All startup weights/queries stream on ONE HWDGE ring in consumption
order: a single queue of large DMAs sustains ~341 GB/s where three
parallel rings drain at only ~280 GB/s aggregate.
"""

import numpy as np

import ml_dtypes

import concourse.bass as bass
import concourse.tile as tile
from concourse import bacc, mybir
from concourse.bass_utils import run_bass_kernel_spmd

B, S, DIN, DH, DOUT = 8, 4096, 512, 1024, 512
SC = 512  # seq chunk processed per pipeline iteration

BF16 = mybir.dt.bfloat16
F32 = mybir.dt.float32


def build_nc():
    nc = bacc.Bacc("TRN2")
    qT = nc.dram_tensor("qT", (DIN, S), BF16, kind="ExternalInput")
    w0t = nc.dram_tensor("w0t", (DIN, DH), BF16, kind="ExternalInput")
    w1t = nc.dram_tensor("w1t", (DH, DH), BF16, kind="ExternalInput")
    w2t = nc.dram_tensor("w2t", (DH, DOUT), BF16, kind="ExternalInput")
    # b0/b1 arrive host-pre-transposed as [128, DH//128] (partition-major):
    # a straight [p, m] load is ~128 descriptors, while a device-side
    # "(m p) -> p m" scatter is ~1024 4-byte descriptors — a descriptor
    # storm that clogs the DMA queue for ~10 us at startup.
    b0 = nc.dram_tensor("b0", (128, DH // 128), F32, kind="ExternalInput")
    b1 = nc.dram_tensor("b1", (128, DH // 128), F32, kind="ExternalInput")
    b2 = nc.dram_tensor("b2", (DOUT,), F32, kind="ExternalInput")
    out = nc.dram_tensor("out", (S, DOUT), BF16, kind="ExternalOutput")

    K0 = DIN // 128   # 4  k-tiles, layer 0
    K1 = DH // 128    # 8  k-tiles, layers 1/2
    M0 = DH // 128    # 8  m-tiles (feature tiles of h0/h1)
    MT = SC // 128    # 4  seq m-tiles per chunk, layer 2
    NCH = S // SC     # 8  chunks

    Relu = mybir.ActivationFunctionType.Relu

    with tile.TileContext(nc) as tc:
        with (
            tc.tile_pool(name="weights", bufs=1) as wpool,
            tc.tile_pool(name="biases", bufs=1) as bpool,
            tc.tile_pool(name="acts", bufs=2) as apool,
            tc.tile_pool(name="qin", bufs=2) as qpool,
            tc.tile_pool(name="outp", bufs=4) as opool,
            # 8 PSUM banks total. L0 gets 3: its groups are only 4 matmuls
            # (864 ns), so with 2 bufs the bank recycle outruns the
            # activation drain and stretches every 8th matmul by ~123 ns.
            # L1/L2 groups are 8 matmuls (1728 ns); L2's consumer (the DVE
            # bias-add, ~500 ns) drains well inside that, so 2 bufs suffice.
            tc.tile_pool(name="psum0", bufs=3, space="PSUM") as ppool0,
            tc.tile_pool(name="psum1", bufs=3, space="PSUM") as ppool1,
            tc.tile_pool(name="psum2", bufs=2, space="PSUM") as ppool2,
        ):
            # Pre-warm the PE clock gate (HAM) with dummy matmuls on garbage
            # data while the startup DMAs land: the real matmul stream then
            # starts at 2.4 GHz.
            g_lhs = apool.tile([128, 128], BF16, tag="warm_lhs")
            g_rhs = apool.tile([128, SC], BF16, tag="warm_rhs")
            nc.vector.memset(g_lhs, 0.0)
            nc.vector.memset(g_rhs, 0.0)
            warm_ps = ppool0.tile([128, SC], F32, tag="ps0")
            # Sized so the warmup block ends right as the first-chunk DMAs
            # land (~14 us): ramping matmuls run ~427-512 ns each.
            N_WARM = 15
            for i in range(N_WARM):
                nc.tensor.matmul(
                    warm_ps, lhsT=g_lhs, rhs=g_rhs,
                    start=(i == 0), stop=(i == N_WARM - 1),
                )

            # Startup loads: ALL on the sync HWDGE ring, in the order the
            # PE consumes them (w0 q0 b0 q1 | w1 b1 | w2 b2). One queue
            # streaming large DMAs sustains ~341 GB/s, while splitting the
            # same bytes across three parallel rings drains at only ~280
            # GB/s aggregate (per-packet queue round-robin overhead) — and
            # FIFO order on one ring guarantees w0/q0 are never slowed by
            # the later, slack-rich w1/w2 transfers. The software pipeline
            # runs L0(c0), L0(c1) (~14 us of PE work) before L1(c0), by
            # which time the 2 MiB of w1 has landed.
            # Weight k-tiles live as the middle dim of one 3D SBUF tile.
            w0_sb = wpool.tile([128, K0, DH], BF16, tag="w0")
            b0_sb = bpool.tile([128, M0], F32, tag="b0")
            b1_sb = bpool.tile([128, M0], F32, tag="b1")
            nc.sync.dma_start(
                out=w0_sb, in_=w0t[:, :].rearrange("(k p) h -> p k h", p=128)
            )

            def w0_slice(k, m):
                return w0_sb[:, k, m * 128:(m + 1) * 128]

            def load_q(c):
                s0 = c * SC
                t = qpool.tile([128, K0, SC], BF16, tag="q", name=f"q{c}")
                nc.sync.dma_start(
                    out=t,
                    in_=qT[:, s0:s0 + SC].rearrange("(k p) s -> p k s", p=128),
                )
                return t

            q0_sb = load_q(0)
            nc.sync.dma_start(out=b0_sb, in_=b0[:, :])
            q1_sb = load_q(1)

            w1_sb = wpool.tile([128, K1, DH], BF16, tag="w1")
            nc.sync.dma_start(
                out=w1_sb, in_=w1t[:, :].rearrange("(k p) h -> p k h", p=128)
            )
            nc.sync.dma_start(out=b1_sb, in_=b1[:, :])

            def w1_slice(k, m):
                return w1_sb[:, k, m * 128:(m + 1) * 128]

            w2_sb = wpool.tile([128, K1, DOUT], BF16, tag="w2")
            nc.sync.dma_start(
                out=w2_sb, in_=w2t[:, :].rearrange("(k p) o -> p k o", p=128)
            )
            b2_sb = bpool.tile([128, DOUT], F32, tag="b2")
            b2_ap = b2[:]
            b2_bcast = bass.AP(
                tensor=b2_ap.tensor,
                offset=b2_ap.offset,
                ap=[[0, 128]] + [list(d) for d in b2_ap.ap],
            )
            nc.sync.dma_start(out=b2_sb, in_=b2_bcast)

            def layer0(c, q_sb):
                h0_sb = []
                for m in range(M0):
                    ps = ppool0.tile([128, SC], F32, tag="ps0", name=f"ps0_{c}_{m}")
                    for k in range(K0):
                        nc.tensor.matmul(
                            ps,
                            lhsT=w0_slice(k, m),
                            rhs=q_sb[:, k, :],
                            start=(k == 0),
                            stop=(k == K0 - 1),
                        )
                    h = apool.tile([128, SC], BF16, tag=f"h0_{m}", name=f"h0_{c}_{m}")
                    nc.scalar.activation(h, ps, Relu, bias=b0_sb[:, m:m + 1])
                    h0_sb.append(h)
                return h0_sb

            def layer1(c, h0_sb):
                h1_sb = []
                for m in range(M0):
                    ps = ppool1.tile([128, SC], F32, tag="ps1", name=f"ps1_{c}_{m}")
                    for k in range(K1):
                        nc.tensor.matmul(
                            ps,
                            lhsT=w1_slice(k, m),
                            rhs=h0_sb[k],
                            start=(k == 0),
                            stop=(k == K1 - 1),
                        )
                    h = apool.tile([128, SC], BF16, tag=f"h1_{m}", name=f"h1_{c}_{m}")
                    nc.scalar.activation(h, ps, Relu, bias=b1_sb[:, m:m + 1])
                    h1_sb.append(h)
                return h1_sb

            def layer2(c, h1_sb):
                s0 = c * SC
                last = c == NCH - 1
                for mt in range(MT):
                    ps = ppool2.tile([128, DOUT], F32, tag="ps2", name=f"ps2_{c}_{mt}")
                    for k in range(K1):
                        nc.tensor.matmul(
                            ps,
                            lhsT=h1_sb[k][:, mt * 128:(mt + 1) * 128],
                            rhs=w2_sb[:, k, :],
                            start=(k == 0),
                            stop=(k == K1 - 1),
                        )
                    ot = opool.tile([128, DOUT], BF16, tag="ot", name=f"ot_{c}_{mt}")
                    r0 = s0 + mt * 128
                    if last and mt == MT - 1:
                        # Tail trim: quarter the strictly-serial
                        # PSUM->add->DMA chain after the very last matmul,
                        # alternating the two HWDGE rings so issue overlaps.
                        Q = DOUT // 4
                        for i in range(4):
                            sl = slice(i * Q, (i + 1) * Q)
                            nc.vector.tensor_add(ot[:, sl], ps[:, sl], b2_sb[:, sl])
                            eng = nc.scalar if i % 2 == 0 else nc.sync
                            eng.dma_start(out=out[r0:r0 + 128, sl], in_=ot[:, sl])
                    else:
                        nc.vector.tensor_add(ot, ps, b2_sb)
                        eng = nc.scalar if mt % 2 == 0 else nc.sync
                        eng.dma_start(out=out[r0:r0 + 128, :], in_=ot)

            # Software pipeline: emit L0 of chunk c+1 ahead of L1/L2 of
            # chunk c, so the matmul stream never depends on a DMA issued
            # less than a full chunk earlier. This also matches the
            # startup-DMA arrival order (w0,q0 | q1 | w1 | w2): L1(c0)
            # only runs after L0(c0)+L0(c1) (~14 us of PE work), by which
            # time the 2 MiB of w1 has landed on the shared HBM port.
            h0_cur = layer0(0, q0_sb)
            for c in range(NCH):
                h0_next = None
                if c + 1 < NCH:
                    h0_next = layer0(c + 1, q1_sb if c == 0 else load_q(c + 1))
                layer2(c, layer1(c, h0_cur))
                h0_cur = h0_next
    nc.finalize()
    return nc


_NC = None


def _get_nc():
    global _NC
    if _NC is None:
        _NC = build_nc()
    return _NC


def make_in_maps(inputs):
    bf16 = ml_dtypes.bfloat16
    q, W0, b0, W1, b1, W2, b2 = (
        inputs["query"], inputs["W0"], inputs["b0"], inputs["W1"],
        inputs["b1"], inputs["W2"], inputs["b2"],
    )
    in_maps = []
    for b in range(B):
        in_maps.append({
            "qT": np.ascontiguousarray(np.asarray(q[b]).T.astype(bf16)),
            "w0t": np.ascontiguousarray(np.asarray(W0[b]).T.astype(bf16)),
            "w1t": np.ascontiguousarray(np.asarray(W1[b]).T.astype(bf16)),
            "w2t": np.ascontiguousarray(np.asarray(W2[b]).T.astype(bf16)),
            "b0": np.ascontiguousarray(
                np.asarray(b0[b], dtype=np.float32).reshape(DH // 128, 128).T
            ),
            "b1": np.ascontiguousarray(
                np.asarray(b1[b], dtype=np.float32).reshape(DH // 128, 128).T
            ),
            "b2": np.asarray(b2[b], dtype=np.float32),
        })
    return in_maps


def run(inputs, trace=False):
    nc = _get_nc()
    in_maps = make_in_maps(inputs)
    res = run_bass_kernel_spmd(nc, in_maps, core_ids=list(range(B)), trace=trace)
    out = np.stack(
        [np.asarray(r["out"]).astype(np.float32) for r in res.results]
    )
    return out, res


def kernel(**inputs) -> np.ndarray:
    out, _ = run(inputs, trace=False)
    return out
